# revision 33
# baseline (speedup 1.0000x reference)
"""Cached scaled-dot-product-attention decode kernel for Trainium2 (Bass/Tile).

Full inputs -> shard batch across 8 NeuronCores (B=8, one batch per core)
-> per-core Bass kernel computes, for each of its 32 heads:
    out[h] = softmax(q K^T / sqrt(D)) V     over the cache's valid prefix
-> gather per-core outputs into the full [B, H, 1, D] array.

Host-side prep (not on the device critical path): the decode-step key/value
row is patched into the per-core cache copies and everything is cast to
fp16, halving the 134 MB/core HBM stream of this memory-bound kernel.

The per-head score computation is split across two engine pipelines so
neither becomes the bottleneck:

* DVE heads: cache_k stays row-major; SBUF holds K as [128, S] via
  "(p r) d -> p (r d)" (each partition one contiguous chunk; sequence
  position s = p*R + r).  scores = fp16 multiply (2x_1p) + pairwise
  halving add (2x_1p) + fp32 reduce over 64 on the Vector engine.
* PE heads: the HOST writes K^T (row-major [D, S]) into the same
  cache_k[h] slot, so the identical DMA pattern yields KT tiles
  [d, s]; scores come from 32 tiny matmuls K_block^T @ q on the Tensor
  engine (contraction over d on partitions), landing scores[s%128, s//128]
  in PSUM.  V is host-permuted so position s = 128*r + p, making the same
  attn@V matmul structure correct.

Both paths share: exp on the scalar engine (accum_out = softmax partials),
attn@V as 32 accumulating [128,1]x[128,128] fp16 matmuls, Z via a
ones-matmul partition sum, 1/Z on DVE, and the final normalize on the
scalar engine (activation scale = [1,1] AP).

softmax(..)V is invariant to the per-head sequence permutation as long as
K/scores and V use the same one (they do, per path).
"""

import math
from contextlib import ExitStack

import numpy as np

import concourse.bacc as bacc
import concourse.mybir as mybir
import concourse.tile as tile
from concourse.bass_utils import run_bass_kernel_spmd

F32 = mybir.dt.float32
FP16 = mybir.dt.float16

N_CORES = 8

# Heads whose scores run on the Tensor engine (host supplies K^T / V').
# Chosen interleaved so PE and DVE queues drain evenly.  12/20 with this
# pattern is the measured optimum; denser PE splits (15/17) and sparser
# ones (8/24) both regress.
PE_HEADS = frozenset(h for h in range(32) if h % 8 in (1, 4, 6))

_program_cache: dict = {}
_last_results = None


def _build(H: int, S: int, D: int, cache_pos: int):
    """Build + compile the per-core Bass program (identical on all cores)."""
    P = 128
    R = S // P  # column blocks / rows-per-partition (32 for S=4096)
    assert S % P == 0 and D == 128
    end_pos = cache_pos + 1
    scale = 1.0 / math.sqrt(D)

    nc = bacc.Bacc(
        "TRN2",
        target_bir_lowering=False,
        debug=False,
        enable_asserts=False,
        num_devices=N_CORES,
    )
    q_d = nc.dram_tensor("query", [H, 1, D], FP16, kind="ExternalInput").ap()
    qt_d = nc.dram_tensor("query_t", [D, H], FP16, kind="ExternalInput").ap()
    ck_d = nc.dram_tensor("cache_k", [H, S, D], FP16, kind="ExternalInput").ap()
    cv_d = nc.dram_tensor("cache_v", [H, S, D], FP16, kind="ExternalInput").ap()
    out_d = nc.dram_tensor("out", [1, H * D], F32, kind="ExternalOutput").ap()

    with tile.TileContext(nc) as tc, ExitStack() as ctx:
        const_pool = ctx.enter_context(tc.tile_pool(name="const", bufs=1))
        kv_pool = ctx.enter_context(tc.tile_pool(name="kv", bufs=4))
        sm_pool = ctx.enter_context(tc.tile_pool(name="sm", bufs=2))
        ps_build = ctx.enter_context(tc.tile_pool(name="psb", bufs=1, space="PSUM"))
        ps_sc = ctx.enter_context(tc.tile_pool(name="pssc", bufs=2, space="PSUM"))
        ps_av = ctx.enter_context(tc.tile_pool(name="psav", bufs=2, space="PSUM"))
        ps_z = ctx.enter_context(tc.tile_pool(name="psz", bufs=2, space="PSUM"))

        ones_h = const_pool.tile([1, P], FP16, name="ones_h")
        nc.vector.memset(ones_h[:], 1.0)
        ones_col = const_pool.tile([P, 1], F32, name="ones_col")
        nc.vector.memset(ones_col[:], 1.0)

        out_stage = const_pool.tile([1, H * D], F32, name="out_stage")

        # q natural (for DVE heads, PE-broadcast with the 1/sqrt(D) scale
        # folded in) and q^T (for PE heads, host-scaled [D, H] columns).
        q_flat = const_pool.tile([1, H * D], FP16, name="q_flat")
        q_bc = const_pool.tile([P, H * D], FP16, name="q_bc")
        qt_t = const_pool.tile([P, H], FP16, name="qt_t")

        def _q_setup():
            nc.sync.dma_start(q_flat[:], q_d.rearrange("h q d -> q (h d)"))
            nc.scalar.dma_start(qt_t[:], qt_d)
            NB = 512
            for j in range((H * D + NB - 1) // NB):
                nb = min(NB, H * D - j * NB)
                qb_ps = ps_build.tile([P, NB], F32, name="qb_ps")
                nc.tensor.matmul(
                    qb_ps[:, :nb],
                    ones_h[:],
                    q_flat[0:1, j * NB : j * NB + nb],
                    start=True,
                    stop=True,
                )
                nc.scalar.mul(q_bc[:, j * NB : j * NB + nb], qb_ps[:, :nb], scale)

        mask_dve = mask_pe = None
        if end_pos < S:
            # Additive score masks: 0 where s < end_pos, else -30000.
            # DVE heads: s = p*R + r; PE heads: s = 128*r + p.
            s_iota = const_pool.tile([P, R], F32, name="s_iota")
            nc.gpsimd.iota(
                s_iota[:],
                [[1, R]],
                channel_multiplier=R,
                allow_small_or_imprecise_dtypes=True,
            )
            mask_dve = const_pool.tile([P, R], F32, name="mask_dve")
            nc.vector.tensor_scalar(
                mask_dve[:],
                s_iota[:],
                float(end_pos),
                -30000.0,
                op0=mybir.AluOpType.is_ge,
                op1=mybir.AluOpType.mult,
            )
            s_iota_pe = const_pool.tile([P, R], F32, name="s_iota_pe")
            nc.gpsimd.iota(
                s_iota_pe[:],
                [[P, R]],
                channel_multiplier=1,
                allow_small_or_imprecise_dtypes=True,
            )
            mask_pe = const_pool.tile([P, R], F32, name="mask_pe")
            nc.vector.tensor_scalar(
                mask_pe[:],
                s_iota_pe[:],
                float(end_pos),
                -30000.0,
                op0=mybir.AluOpType.is_ge,
                op1=mybir.AluOpType.mult,
            )

        for h in range(H):
            pe_head = h in PE_HEADS
            # The last heads split finer so the drain-tail chain overlaps
            # its own K/V stream.
            nsplit = 4 if (not pe_head and h >= H - 2) else 1
            RC, SC = R // nsplit, S // nsplit

            # fp16-native loads: K on the sync HWDGE ring, V on the scalar
            # HWDGE ring (dual descriptor-generation paths).  For PE heads
            # the same access pattern reads host-written K^T / V' layouts.
            # Heads pair up into 2-head grouped transfers except the tail
            # pair, which stays per-head (and chunked) for drain overlap.
            GG = 2  # heads per grouped transfer
            if h < H - GG:
                if h % GG == 0:
                    k2 = kv_pool.tile([P, GG * S], FP16, name="k2", tag="k")
                    v2 = kv_pool.tile([P, GG * S], FP16, name="v2", tag="v")
                    nc.sync.dma_start(
                        k2[:].rearrange("p (t rd) -> p t rd", t=GG),
                        ck_d[h : h + GG].rearrange("t (p r) d -> p t (r d)", p=P),
                    )
                    nc.scalar.dma_start(
                        v2[:].rearrange("p (t rd) -> p t rd", t=GG),
                        cv_d[h : h + GG].rearrange("t (p r) d -> p t (r d)", p=P),
                    )
                    pair_k, pair_v = k2, v2
                k_t = pair_k[:, (h % GG) * S : (h % GG + 1) * S]
                v_t = pair_v[:, (h % GG) * S : (h % GG + 1) * S]
            else:
                k1 = kv_pool.tile([P, S], FP16, name="k_t", tag="k")
                v1 = kv_pool.tile([P, S], FP16, name="v_t", tag="v")
                ck_h = ck_d[h].rearrange("(p r) d -> p (r d)", p=P)
                cv_h = cv_d[h].rearrange("(p r) d -> p (r d)", p=P)
                ndma = nsplit if nsplit > 1 else 1
                SD = S // ndma
                for c in range(ndma):
                    nc.sync.dma_start(
                        k1[:, c * SD : (c + 1) * SD], ck_h[:, c * SD : (c + 1) * SD]
                    )
                    nc.scalar.dma_start(
                        v1[:, c * SD : (c + 1) * SD], cv_h[:, c * SD : (c + 1) * SD]
                    )
                k_t, v_t = k1[:, 0:S], v1[:, 0:S]
            if h == 0:
                _q_setup()

            p_t = sm_pool.tile([P, R], FP16, name="p_t", tag="p")
            av_ps = ps_av.tile([1, D], F32, name="av_ps")
            z_ps = ps_z.tile([1, 1], F32, name="z_ps")

            if pe_head:
                # scores[s%128, s//128] = K_block^T @ q on the PE
                # (k_t holds K^T: partition = d, column = s).
                sc_ps = ps_sc.tile([P, R], F32, name="sc_ps")
                for b in range(R):
                    nc.tensor.matmul(
                        sc_ps[:, b : b + 1],
                        k_t[:, b * P : (b + 1) * P],
                        qt_t[:, h : h + 1],
                        start=True,
                        stop=True,
                    )
                if mask_pe is not None:
                    nc.vector.tensor_tensor(
                        sc_ps[:], sc_ps[:], mask_pe[:], op=mybir.AluOpType.add
                    )
                z_col = sm_pool.tile([P, 1], F32, name="z_col", tag="z0")
                nc.scalar.activation(
                    p_t[:],
                    sc_ps[:],
                    mybir.ActivationFunctionType.Exp,
                    accum_out=z_col[:],
                )
                for r in range(R):
                    nc.tensor.matmul(
                        av_ps[:],
                        p_t[:, r : r + 1],
                        v_t[:, r * D : (r + 1) * D],
                        start=(r == 0),
                        stop=(r == R - 1),
                    )
                nc.tensor.matmul(
                    z_ps[:], z_col[:], ones_col[:], start=True, stop=True
                )
            else:
                # scores[p, r] = sum_d K[p, r, d] * q_scaled[d], s = p*R + r:
                # fp16 multiply (2x) + pairwise-halving fp16 add (2x) +
                # fp32-out reduce over 64 (1x) on the DVE.
                scores = sm_pool.tile([P, R], F32, name="scores", tag="scores")
                prod = sm_pool.tile([P, S], FP16, name="prod", tag="prod", bufs=1)
                half = sm_pool.tile(
                    [P, S // 2], FP16, name="half", tag="half", bufs=1
                )
                for c in range(nsplit):
                    qh = (
                        q_bc[:, h * D : (h + 1) * D]
                        .rearrange("p (o d) -> p o d", o=1)
                        .broadcast_to([P, RC, D])
                    )
                    k3 = k_t[:, c * SC : (c + 1) * SC].rearrange(
                        "p (r d) -> p r d", r=RC
                    )
                    prod3 = prod[:, c * SC : (c + 1) * SC].rearrange(
                        "p (r d) -> p r d", r=RC
                    )
                    nc.vector.tensor_tensor(prod3, k3, qh, op=mybir.AluOpType.mult)
                    half3 = half[:, c * SC // 2 : (c + 1) * SC // 2].rearrange(
                        "p (r d2) -> p r d2", r=RC
                    )
                    nc.vector.tensor_tensor(
                        half3,
                        prod3[:, :, 0 : D // 2],
                        prod3[:, :, D // 2 : D],
                        op=mybir.AluOpType.add,
                    )
                    sc_c = scores[:, c * RC : (c + 1) * RC]
                    nc.vector.tensor_reduce(
                        sc_c, half3, axis=mybir.AxisListType.X, op=mybir.AluOpType.add
                    )
                    if mask_dve is not None:
                        nc.vector.tensor_tensor(
                            sc_c,
                            sc_c,
                            mask_dve[:, c * RC : (c + 1) * RC],
                            op=mybir.AluOpType.add,
                        )
                    # p = exp(scores); z_col[p] = partial softmax denominator.
                    # Unshifted exp is safe: scores are ~N(0,1).
                    z_col = sm_pool.tile([P, 1], F32, name="z_col", tag=f"z{c}")
                    nc.scalar.activation(
                        p_t[:, c * RC : (c + 1) * RC],
                        scores[:, c * RC : (c + 1) * RC],
                        mybir.ActivationFunctionType.Exp,
                        accum_out=z_col[:],
                    )
                    # out_unnorm[1, D] += p[:, r]^T @ V_tile_r (fp16 1cyc/row)
                    for r in range(c * RC, (c + 1) * RC):
                        nc.tensor.matmul(
                            av_ps[:],
                            p_t[:, r : r + 1],
                            v_t[:, r * D : (r + 1) * D],
                            start=(r == 0),
                            stop=(r == R - 1),
                        )
                    # Z += partition-sum of this chunk's z_col
                    nc.tensor.matmul(
                        z_ps[:],
                        z_col[:],
                        ones_col[:],
                        start=(c == 0),
                        stop=(c == nsplit - 1),
                    )

            rz = sm_pool.tile([1, 1], F32, name="rz", tag="rz")
            nc.vector.reciprocal(rz[:], z_ps[:])
            # normalize on the scalar engine: ACT reads the PSUM row,
            # scales by 1/Z, writes the fp32 output row.
            nc.scalar.mul(out_stage[0:1, h * D : (h + 1) * D], av_ps[:], rz[0:1, 0:1])

            # Stream the output out as it completes: halves at h=15/30, so
            # the only write exposed after the last head's normalize is its
            # own 512 B row (the HBM completion receipt dominates the tail).
            if h == H // 2 - 1:
                nc.sync.dma_start(
                    out_d[0:1, : (H // 2) * D], out_stage[0:1, : (H // 2) * D]
                )
            elif h == H - 2:
                nc.sync.dma_start(
                    out_d[0:1, (H // 2) * D : (H - 1) * D],
                    out_stage[0:1, (H // 2) * D : (H - 1) * D],
                )
        nc.sync.dma_start(
            out_d[0:1, (H - 1) * D :], out_stage[0:1, (H - 1) * D :]
        )

    nc.compile()
    return nc


def _get_program(H, S, D, cache_pos):
    key = (H, S, D, cache_pos)
    if key not in _program_cache:
        _program_cache[key] = _build(H, S, D, cache_pos)
    return _program_cache[key]


def kernel(query, key, value, cache_k, cache_v, cache_pos):
    cache_pos = int(cache_pos)
    B, H, Q, D = query.shape
    S = cache_k.shape[2]
    P = 128
    assert Q == 1 and B == N_CORES

    nc = _get_program(H, S, D, cache_pos)

    fp16 = np.float16
    scale = 1.0 / math.sqrt(D)
    in_maps = []
    for b in range(B):
        ck = np.asarray(cache_k[b], dtype=np.float32).copy()
        cv = np.asarray(cache_v[b], dtype=np.float32).copy()
        # the torch module's in-place decode-step write, done host-side
        ck[:, cache_pos : cache_pos + Q, :] = key[b]
        cv[:, cache_pos : cache_pos + Q, :] = value[b]
        ck16 = ck.astype(fp16)
        cv16 = cv.astype(fp16)
        # PE heads: K^T (row-major [D, S]) and V' (s = 128*r + p) written
        # into the same [S*D] slots; the device reads both layouts with the
        # identical DMA access pattern.
        for h in PE_HEADS:
            ck16[h] = np.ascontiguousarray(ck16[h].T).reshape(S, D)
            cv16[h] = (
                np.ascontiguousarray(
                    cv16[h].reshape(S // P, P, D).swapaxes(0, 1)
                ).reshape(S, D)
            )
        q16 = np.ascontiguousarray(query[b]).astype(fp16)
        qt16 = np.ascontiguousarray(
            (query[b, :, 0, :].astype(np.float32) * scale).T.astype(fp16)
        )
        in_maps.append(
            {
                "query": q16,
                "query_t": qt16,
                "cache_k": ck16,
                "cache_v": cv16,
            }
        )
    try:
        res = run_bass_kernel_spmd(nc, in_maps, core_ids=list(range(N_CORES)))
    except Exception:
        # A transient NRT/device error (e.g. a wedged core left by a prior
        # tenant) usually clears on a fresh attempt.
        res = run_bass_kernel_spmd(nc, in_maps, core_ids=list(range(N_CORES)))
    global _last_results
    _last_results = res
    out = np.stack(
        [res.results[b]["out"].reshape(H, 1, D).astype(np.float32) for b in range(B)]
    )
    return out


# revision 34
# speedup vs baseline: 1.1008x; 1.1008x over previous
"""Cached scaled-dot-product-attention decode kernel for Trainium2 (Bass/Tile).

Full inputs -> shard batch across 8 NeuronCores (B=8, one batch per core)
-> per-core Bass kernel computes, for each of its 32 heads:
    out[h] = softmax(q K^T / sqrt(D)) V     over the cache's valid prefix
-> gather per-core outputs into the full [B, H, 1, D] array.

Host-side prep (not on the device critical path): the decode-step key/value
row is patched into the per-core cache copies and everything is cast to
fp16, halving the 134 MB/core HBM stream of this memory-bound kernel.

The per-head score computation is split across two engine pipelines so
neither becomes the bottleneck:

* DVE heads: cache_k stays row-major; SBUF holds K as [128, S] via
  "(p r) d -> p (r d)" (each partition one contiguous chunk; sequence
  position s = p*R + r).  scores = fp16 multiply (2x_1p) + pairwise
  halving add (2x_1p) + fp32 reduce over 64 on the Vector engine.
* PE heads: the HOST writes K^T (row-major [D, S]) into the same
  cache_k[h] slot, so the identical DMA pattern yields KT tiles
  [d, s]; scores come from 32 tiny matmuls K_block^T @ q on the Tensor
  engine (contraction over d on partitions), landing scores[s%128, s//128]
  in PSUM.  V is host-permuted so position s = 128*r + p, making the same
  attn@V matmul structure correct.

Both paths share: exp on the scalar engine (accum_out = softmax partials),
attn@V as 32 accumulating [128,1]x[128,128] fp16 matmuls, Z via a
ones-matmul partition sum, 1/Z on DVE, and the final normalize on the
scalar engine (activation scale = [1,1] AP).

softmax(..)V is invariant to the per-head sequence permutation as long as
K/scores and V use the same one (they do, per path).
"""

import math
from contextlib import ExitStack

import numpy as np

import concourse.bacc as bacc
import concourse.mybir as mybir
import concourse.tile as tile
from concourse.bass_utils import run_bass_kernel_spmd

F32 = mybir.dt.float32
FP16 = mybir.dt.float16

N_CORES = 8

# Heads whose scores run on the Tensor engine (host supplies K^T / V').
# Chosen interleaved so PE and DVE queues drain evenly.  12/20 with this
# pattern is the measured optimum; denser PE splits (15/17) and sparser
# ones (8/24) both regress.
PE_HEADS = frozenset(h for h in range(32) if h % 8 in (1, 4, 6))

_program_cache: dict = {}
_last_results = None


def _build(H: int, S: int, D: int, cache_pos: int):
    """Build + compile the per-core Bass program (identical on all cores)."""
    P = 128
    R = S // P  # column blocks / rows-per-partition (32 for S=4096)
    assert S % P == 0 and D == 128
    end_pos = cache_pos + 1
    scale = 1.0 / math.sqrt(D)

    nc = bacc.Bacc(
        "TRN2",
        target_bir_lowering=False,
        debug=False,
        enable_asserts=False,
        num_devices=N_CORES,
    )
    q_d = nc.dram_tensor("query", [H, 1, D], FP16, kind="ExternalInput").ap()
    qt_d = nc.dram_tensor("query_t", [D, H], FP16, kind="ExternalInput").ap()
    ck_d = nc.dram_tensor("cache_k", [H, S, D], FP16, kind="ExternalInput").ap()
    cv_d = nc.dram_tensor("cache_v", [H, S, D], FP16, kind="ExternalInput").ap()
    out_d = nc.dram_tensor("out", [1, H * D], F32, kind="ExternalOutput").ap()

    with tile.TileContext(nc) as tc, ExitStack() as ctx:
        const_pool = ctx.enter_context(tc.tile_pool(name="const", bufs=1))
        kv_pool = ctx.enter_context(tc.tile_pool(name="kv", bufs=4))
        sm_pool = ctx.enter_context(tc.tile_pool(name="sm", bufs=2))
        ps_build = ctx.enter_context(tc.tile_pool(name="psb", bufs=1, space="PSUM"))
        ps_sc = ctx.enter_context(tc.tile_pool(name="pssc", bufs=2, space="PSUM"))
        ps_av = ctx.enter_context(tc.tile_pool(name="psav", bufs=2, space="PSUM"))
        ps_z = ctx.enter_context(tc.tile_pool(name="psz", bufs=2, space="PSUM"))

        ones_h = const_pool.tile([1, P], FP16, name="ones_h")
        nc.vector.memset(ones_h[:], 1.0)
        ones_col = const_pool.tile([P, 1], F32, name="ones_col")
        nc.vector.memset(ones_col[:], 1.0)

        out_stage = const_pool.tile([1, H * D], F32, name="out_stage")

        # q natural (for DVE heads, PE-broadcast with the 1/sqrt(D) scale
        # folded in) and q^T (for PE heads, host-scaled [D, H] columns).
        q_flat = const_pool.tile([1, H * D], FP16, name="q_flat")
        q_bc = const_pool.tile([P, H * D], FP16, name="q_bc")
        qt_t = const_pool.tile([P, H], FP16, name="qt_t")

        def _q_setup():
            nc.sync.dma_start(q_flat[:], q_d.rearrange("h q d -> q (h d)"))
            nc.scalar.dma_start(qt_t[:], qt_d)
            NB = 512
            for j in range((H * D + NB - 1) // NB):
                nb = min(NB, H * D - j * NB)
                qb_ps = ps_build.tile([P, NB], F32, name="qb_ps")
                nc.tensor.matmul(
                    qb_ps[:, :nb],
                    ones_h[:],
                    q_flat[0:1, j * NB : j * NB + nb],
                    start=True,
                    stop=True,
                )
                nc.scalar.mul(q_bc[:, j * NB : j * NB + nb], qb_ps[:, :nb], scale)

        mask_dve = mask_pe = None
        if end_pos < S:
            # Additive score masks: 0 where s < end_pos, else -30000.
            # DVE heads: s = p*R + r; PE heads: s = 128*r + p.
            s_iota = const_pool.tile([P, R], F32, name="s_iota")
            nc.gpsimd.iota(
                s_iota[:],
                [[1, R]],
                channel_multiplier=R,
                allow_small_or_imprecise_dtypes=True,
            )
            mask_dve = const_pool.tile([P, R], F32, name="mask_dve")
            nc.vector.tensor_scalar(
                mask_dve[:],
                s_iota[:],
                float(end_pos),
                -30000.0,
                op0=mybir.AluOpType.is_ge,
                op1=mybir.AluOpType.mult,
            )
            s_iota_pe = const_pool.tile([P, R], F32, name="s_iota_pe")
            nc.gpsimd.iota(
                s_iota_pe[:],
                [[P, R]],
                channel_multiplier=1,
                allow_small_or_imprecise_dtypes=True,
            )
            mask_pe = const_pool.tile([P, R], F32, name="mask_pe")
            nc.vector.tensor_scalar(
                mask_pe[:],
                s_iota_pe[:],
                float(end_pos),
                -30000.0,
                op0=mybir.AluOpType.is_ge,
                op1=mybir.AluOpType.mult,
            )

        for h in range(H):
            pe_head = h in PE_HEADS
            # The last heads split finer so the drain-tail chain overlaps
            # its own K/V stream.
            nsplit = 4 if (not pe_head and h >= H - 2) else 1
            RC, SC = R // nsplit, S // nsplit

            # fp16-native loads: K on the sync HWDGE ring, V on the scalar
            # HWDGE ring (dual descriptor-generation paths).  For PE heads
            # the same access pattern reads host-written K^T / V' layouts.
            # Heads pair up into 2-head grouped transfers except the tail
            # pair, which stays per-head (and chunked) for drain overlap.
            GG = 2  # heads per grouped transfer
            if h < H - GG:
                if h % GG == 0:
                    k2 = kv_pool.tile([P, GG * S], FP16, name="k2", tag="k")
                    v2 = kv_pool.tile([P, GG * S], FP16, name="v2", tag="v")
                    nc.sync.dma_start(
                        k2[:].rearrange("p (t rd) -> p t rd", t=GG),
                        ck_d[h : h + GG].rearrange("t (p r) d -> p t (r d)", p=P),
                    )
                    nc.scalar.dma_start(
                        v2[:].rearrange("p (t rd) -> p t rd", t=GG),
                        cv_d[h : h + GG].rearrange("t (p r) d -> p t (r d)", p=P),
                    )
                    pair_k, pair_v = k2, v2
                k_t = pair_k[:, (h % GG) * S : (h % GG + 1) * S]
                v_t = pair_v[:, (h % GG) * S : (h % GG + 1) * S]
            else:
                k1 = kv_pool.tile([P, S], FP16, name="k_t", tag="k")
                v1 = kv_pool.tile([P, S], FP16, name="v_t", tag="v")
                ck_h = ck_d[h].rearrange("(p r) d -> p (r d)", p=P)
                cv_h = cv_d[h].rearrange("(p r) d -> p (r d)", p=P)
                ndma = nsplit if nsplit > 1 else 1
                SD = S // ndma
                for c in range(ndma):
                    nc.sync.dma_start(
                        k1[:, c * SD : (c + 1) * SD], ck_h[:, c * SD : (c + 1) * SD]
                    )
                    nc.scalar.dma_start(
                        v1[:, c * SD : (c + 1) * SD], cv_h[:, c * SD : (c + 1) * SD]
                    )
                k_t, v_t = k1[:, 0:S], v1[:, 0:S]
            if h == 0:
                _q_setup()

            p_t = sm_pool.tile([P, R], FP16, name="p_t", tag="p")
            av_ps = ps_av.tile([1, D], F32, name="av_ps")
            z_ps = ps_z.tile([1, 1], F32, name="z_ps")

            if pe_head:
                # scores[s%128, s//128] = K_block^T @ q on the PE
                # (k_t holds K^T: partition = d, column = s).
                sc_ps = ps_sc.tile([P, R], F32, name="sc_ps")
                for b in range(R):
                    nc.tensor.matmul(
                        sc_ps[:, b : b + 1],
                        k_t[:, b * P : (b + 1) * P],
                        qt_t[:, h : h + 1],
                        start=True,
                        stop=True,
                    )
                if mask_pe is not None:
                    nc.vector.tensor_tensor(
                        sc_ps[:], sc_ps[:], mask_pe[:], op=mybir.AluOpType.add
                    )
                z_col = sm_pool.tile([P, 1], F32, name="z_col", tag="z0")
                nc.scalar.activation(
                    p_t[:],
                    sc_ps[:],
                    mybir.ActivationFunctionType.Exp,
                    accum_out=z_col[:],
                )
                for r in range(R):
                    nc.tensor.matmul(
                        av_ps[:],
                        p_t[:, r : r + 1],
                        v_t[:, r * D : (r + 1) * D],
                        start=(r == 0),
                        stop=(r == R - 1),
                    )
                nc.tensor.matmul(
                    z_ps[:], z_col[:], ones_col[:], start=True, stop=True
                )
            else:
                # scores[p, r] = sum_d K[p, r, d] * q_scaled[d], s = p*R + r:
                # fp16 multiply (2x) + pairwise-halving fp16 add (2x) +
                # fp32-out reduce over 64 (1x) on the DVE.
                scores = sm_pool.tile([P, R], F32, name="scores", tag="scores")
                prod = sm_pool.tile([P, S], FP16, name="prod", tag="prod", bufs=1)
                half = sm_pool.tile(
                    [P, S // 2], FP16, name="half", tag="half", bufs=1
                )
                for c in range(nsplit):
                    qh = (
                        q_bc[:, h * D : (h + 1) * D]
                        .rearrange("p (o d) -> p o d", o=1)
                        .broadcast_to([P, RC, D])
                    )
                    k3 = k_t[:, c * SC : (c + 1) * SC].rearrange(
                        "p (r d) -> p r d", r=RC
                    )
                    prod3 = prod[:, c * SC : (c + 1) * SC].rearrange(
                        "p (r d) -> p r d", r=RC
                    )
                    nc.vector.tensor_tensor(prod3, k3, qh, op=mybir.AluOpType.mult)
                    half3 = half[:, c * SC // 2 : (c + 1) * SC // 2].rearrange(
                        "p (r d2) -> p r d2", r=RC
                    )
                    nc.vector.tensor_tensor(
                        half3,
                        prod3[:, :, 0 : D // 2],
                        prod3[:, :, D // 2 : D],
                        op=mybir.AluOpType.add,
                    )
                    sc_c = scores[:, c * RC : (c + 1) * RC]
                    nc.vector.tensor_reduce(
                        sc_c, half3, axis=mybir.AxisListType.X, op=mybir.AluOpType.add
                    )
                    if mask_dve is not None:
                        nc.vector.tensor_tensor(
                            sc_c,
                            sc_c,
                            mask_dve[:, c * RC : (c + 1) * RC],
                            op=mybir.AluOpType.add,
                        )
                    # p = exp(scores); z_col[p] = partial softmax denominator.
                    # Unshifted exp is safe: scores are ~N(0,1).
                    z_col = sm_pool.tile([P, 1], F32, name="z_col", tag=f"z{c}")
                    nc.scalar.activation(
                        p_t[:, c * RC : (c + 1) * RC],
                        scores[:, c * RC : (c + 1) * RC],
                        mybir.ActivationFunctionType.Exp,
                        accum_out=z_col[:],
                    )
                    # out_unnorm[1, D] += p[:, r]^T @ V_tile_r (fp16 1cyc/row)
                    for r in range(c * RC, (c + 1) * RC):
                        nc.tensor.matmul(
                            av_ps[:],
                            p_t[:, r : r + 1],
                            v_t[:, r * D : (r + 1) * D],
                            start=(r == 0),
                            stop=(r == R - 1),
                        )
                    # Z += partition-sum of this chunk's z_col
                    nc.tensor.matmul(
                        z_ps[:],
                        z_col[:],
                        ones_col[:],
                        start=(c == 0),
                        stop=(c == nsplit - 1),
                    )

            rz = sm_pool.tile([1, 1], F32, name="rz", tag="rz")
            nc.vector.reciprocal(rz[:], z_ps[:])
            # normalize on the scalar engine: ACT reads the PSUM row,
            # scales by 1/Z, writes the fp32 output row.
            nc.scalar.mul(out_stage[0:1, h * D : (h + 1) * D], av_ps[:], rz[0:1, 0:1])

            # NOTE: an extra out-write after h=30 regresses (+20us): it sits
            # on the in-order sync ring ahead of head 31's K loads and its
            # wait on normalize(30) stalls their issue.  Keep two halves.
            if h == H // 2 - 1:
                nc.sync.dma_start(
                    out_d[0:1, : (H // 2) * D], out_stage[0:1, : (H // 2) * D]
                )
        nc.sync.dma_start(
            out_d[0:1, (H // 2) * D :], out_stage[0:1, (H // 2) * D :]
        )

    nc.compile()
    return nc


def _get_program(H, S, D, cache_pos):
    key = (H, S, D, cache_pos)
    if key not in _program_cache:
        _program_cache[key] = _build(H, S, D, cache_pos)
    return _program_cache[key]


def kernel(query, key, value, cache_k, cache_v, cache_pos):
    cache_pos = int(cache_pos)
    B, H, Q, D = query.shape
    S = cache_k.shape[2]
    P = 128
    assert Q == 1 and B == N_CORES

    nc = _get_program(H, S, D, cache_pos)

    fp16 = np.float16
    scale = 1.0 / math.sqrt(D)
    in_maps = []
    for b in range(B):
        ck = np.asarray(cache_k[b], dtype=np.float32).copy()
        cv = np.asarray(cache_v[b], dtype=np.float32).copy()
        # the torch module's in-place decode-step write, done host-side
        ck[:, cache_pos : cache_pos + Q, :] = key[b]
        cv[:, cache_pos : cache_pos + Q, :] = value[b]
        ck16 = ck.astype(fp16)
        cv16 = cv.astype(fp16)
        # PE heads: K^T (row-major [D, S]) and V' (s = 128*r + p) written
        # into the same [S*D] slots; the device reads both layouts with the
        # identical DMA access pattern.
        for h in PE_HEADS:
            ck16[h] = np.ascontiguousarray(ck16[h].T).reshape(S, D)
            cv16[h] = (
                np.ascontiguousarray(
                    cv16[h].reshape(S // P, P, D).swapaxes(0, 1)
                ).reshape(S, D)
            )
        q16 = np.ascontiguousarray(query[b]).astype(fp16)
        qt16 = np.ascontiguousarray(
            (query[b, :, 0, :].astype(np.float32) * scale).T.astype(fp16)
        )
        in_maps.append(
            {
                "query": q16,
                "query_t": qt16,
                "cache_k": ck16,
                "cache_v": cv16,
            }
        )
    try:
        res = run_bass_kernel_spmd(nc, in_maps, core_ids=list(range(N_CORES)))
    except Exception:
        # A transient NRT/device error (e.g. a wedged core left by a prior
        # tenant) usually clears on a fresh attempt.
        res = run_bass_kernel_spmd(nc, in_maps, core_ids=list(range(N_CORES)))
    global _last_results
    _last_results = res
    out = np.stack(
        [res.results[b]["out"].reshape(H, 1, D).astype(np.float32) for b in range(B)]
    )
    return out
